# revision 1
# baseline (speedup 1.0000x reference)
"""Trainium2 Bass kernel for the AdapterController hard-routing MoE adapter.

Reference computation (per router m in [0,4), batch b in [0,16)):
    e = expert_index[m, b]
    z = x[b] @ down_w[m, e] + down_b[m, e]      # [512, 256]
    z = z * sigmoid(z)                          # swish
    u = z @ up_w[m, e]                          # [512, 1024]
    out[m, b] = u

Strategy: data-parallel over the batch axis (2 batches per core, 8 cores).
The expert gather is part of input sharding: each core receives exactly the
(m, b)-selected weight matrices, packed on the host into the SBUF partition
layout so every DMA is a single fully-contiguous >=1MB transfer.

On-chip per (m, b) pair (all matmuls in float32r = full-rate fp32):
    zT[d, s] = sum_c Wd[c, d] * xT[c, s]        (16 matmuls N=512, K=128)
    z = silu(zT + bd)                           (ACT engine, PSUM -> SBUF)
    u[s, c] = sum_d zT[d, s].T @ Wu[d, c]       (16 matmuls N=512)
"""

import numpy as np

M, B, S, C, D = 4, 16, 512, 1024, 256
N_CORES = 8
B_LOC = B // N_CORES  # batches per core
KC = C // 128         # 8 c-chunks
KD = D // 128         # 2 d-chunks
KS = S // 128         # 4 s-chunks

_cache = {}
last_results = None  # BassKernelResults of the most recent run (for test.py)


def _build():
    from contextlib import ExitStack

    import concourse.mybir as mybir
    import concourse.tile as tile
    from concourse import bacc

    f32 = mybir.dt.float32
    f32r = mybir.dt.float32r

    nc = bacc.Bacc("TRN2", target_bir_lowering=False, debug=False,
                   num_devices=N_CORES)
    # xtp[b][p, k*512 + s] = x[b, s, 128k + p]   (x transposed, chunk-packed)
    xtp = nc.dram_tensor("xtp", [B_LOC, 128, KC * S], f32,
                         kind="ExternalInput").ap()
    # wpk[m, b][p, k*256 + d]        = down_w_gathered[m, b, 128k + p, d]
    # wpk[m, b][p, 2048 + j*1024+c]  = up_w_gathered[m, b, 128j + p, c]
    wpk = nc.dram_tensor("wpk", [M, B_LOC, 128, 4096], f32,
                         kind="ExternalInput").ap()
    # bdp[p, (m*B_LOC+b)*2 + j] = down_b_gathered[m, b, 128j + p]
    bdp = nc.dram_tensor("bdp", [128, M * B_LOC * KD], f32,
                         kind="ExternalInput").ap()
    out = nc.dram_tensor("out", [M, B_LOC, S, C], f32,
                         kind="ExternalOutput").ap()

    silu = mybir.ActivationFunctionType.Silu

    with tile.TileContext(nc) as tc, ExitStack() as ctx:
        const = ctx.enter_context(tc.tile_pool(name="const", bufs=1))
        xpool = ctx.enter_context(tc.tile_pool(name="xpool", bufs=2))
        wpool = ctx.enter_context(tc.tile_pool(name="wpool", bufs=2))
        zpool = ctx.enter_context(tc.tile_pool(name="zpool", bufs=2))
        upool = ctx.enter_context(tc.tile_pool(name="upool", bufs=2))
        pszp = ctx.enter_context(tc.tile_pool(name="pszp", bufs=2, space="PSUM"))
        psup = ctx.enter_context(tc.tile_pool(name="psup", bufs=4, space="PSUM"))

        bd_sb = const.tile([128, M * B_LOC * KD], f32)
        nc.sync.dma_start(bd_sb[:], bdp[:])

        for b in range(B_LOC):
            xt_sb = xpool.tile([128, KC * S], f32r)
            nc.gpsimd.dma_start(xt_sb[:], xtp[b])  # casting DMA f32 -> f32r
            for m in range(M):
                w_sb = wpool.tile([128, 4096], f32r)
                nc.gpsimd.dma_start(w_sb[:], wpk[m, b])

                z_sb = zpool.tile([128, KD, S], f32r)
                for j in range(KD):
                    psz = pszp.tile([128, S], f32)
                    for k in range(KC):
                        nc.tensor.matmul(
                            psz[:],
                            w_sb[:, k * 256 + j * 128: k * 256 + j * 128 + 128],
                            xt_sb[:, k * S: (k + 1) * S],
                            start=(k == 0), stop=(k == KC - 1),
                        )
                    col = (m * B_LOC + b) * KD + j
                    nc.scalar.activation(z_sb[:, j, :], psz[:], silu,
                                         bias=bd_sb[:, col: col + 1])

                u_sb = upool.tile([128, KS, C], f32)
                for a in range(KS):
                    for h in range(2):
                        psu = psup.tile([128, 512], f32)
                        for j in range(KD):
                            nc.tensor.matmul(
                                psu[:],
                                z_sb[:, j, a * 128: (a + 1) * 128],
                                w_sb[:, 2048 + j * 1024 + h * 512:
                                     2048 + j * 1024 + h * 512 + 512],
                                start=(j == 0), stop=(j == KD - 1),
                            )
                        dst = u_sb[:, a, h * 512: (h + 1) * 512]
                        if (a * 2 + h) % 2 == 0:
                            nc.vector.tensor_copy(dst, psu[:])
                        else:
                            nc.scalar.activation(
                                dst, psu[:], mybir.ActivationFunctionType.Copy)
                nc.sync.dma_start(
                    out[m, b].rearrange("(a p) c -> p a c", p=128), u_sb[:])

    nc.compile()
    return nc


def _get_nc():
    if "nc" not in _cache:
        _cache["nc"] = _build()
    return _cache["nc"]


def kernel(x, expert_index, down_w, down_b, up_w):
    global last_results
    from concourse import bass_utils

    x = np.asarray(x, dtype=np.float32)
    idx = np.asarray(expert_index)
    r = np.arange(M)[:, None]
    wd = np.asarray(down_w, dtype=np.float32)[r, idx]   # [M, B, C, D]
    bd = np.asarray(down_b, dtype=np.float32)[r, idx]   # [M, B, D]
    wu = np.asarray(up_w, dtype=np.float32)[r, idx]     # [M, B, D, C]

    # Pack into SBUF partition-major layouts (see _build comments).
    xt = x.transpose(0, 2, 1).reshape(B, KC, 128, S)
    xt = xt.transpose(0, 2, 1, 3).reshape(B, 128, KC * S)
    wdp = wd.reshape(M, B, KC, 128, D).transpose(0, 1, 3, 2, 4)
    wdp = wdp.reshape(M, B, 128, KC * D)
    wup = wu.reshape(M, B, KD, 128, C).transpose(0, 1, 3, 2, 4)
    wup = wup.reshape(M, B, 128, KD * C)
    wpk = np.concatenate([wdp, wup], axis=-1)           # [M, B, 128, 4096]
    bdp = bd.reshape(M, B, KD, 128).transpose(3, 0, 1, 2)
    bdp = bdp.reshape(128, M * B * KD)                  # [128, M*B*KD]

    in_maps = []
    for i in range(N_CORES):
        bs = slice(i * B_LOC, (i + 1) * B_LOC)
        cols = bdp.reshape(128, M, B, KD)[:, :, bs, :].reshape(128, M * B_LOC * KD)
        in_maps.append({
            "xtp": np.ascontiguousarray(xt[bs]),
            "wpk": np.ascontiguousarray(wpk[:, bs]),
            "bdp": np.ascontiguousarray(cols),
        })

    nc = _get_nc()
    res = bass_utils.run_bass_kernel_spmd(nc, in_maps,
                                          core_ids=list(range(N_CORES)))
    last_results = res

    full = np.empty((M, B, S, C), dtype=np.float32)
    for i in range(N_CORES):
        full[:, i * B_LOC:(i + 1) * B_LOC] = res.results[i]["out"]
    return full


# revision 3
# speedup vs baseline: 1.3844x; 1.3844x over previous
"""Trainium2 Bass kernel for the AdapterController hard-routing MoE adapter.

Reference computation (per router m in [0,4), batch b in [0,16)):
    e = expert_index[m, b]
    z = x[b] @ down_w[m, e] + down_b[m, e]      # [512, 256]
    z = z * sigmoid(z)                          # swish
    u = z @ up_w[m, e]                          # [512, 1024]
    out[m, b] = u

Strategy: data-parallel over the batch axis (2 batches per core, 8 cores).
The expert gather is part of input sharding: each core receives exactly the
(m, b)-selected weight matrices, packed on the host into the SBUF partition
layout so every DMA is a single fully-contiguous >=1MB transfer.

On-chip per (m, b) pair:
    zT[d, s] = sum_c Wd[c, d] * xT[c, s]        (16 matmuls N=512, K=128)
    z = silu(zT + bd)                           (ACT engine, PSUM -> SBUF)
    u[s, c] = sum_d zT[d, s].T @ Wu[d, c]       (16 matmuls N=512)

MODE selects compute/transfer dtypes:
    "f32r":  f32 DMA, float32r matmuls (TF32-like, full PE rate)
    "bf16":  bf16 weights/x (host-cast), f32 output
    "bf16o": bf16 weights/x and bf16 output (host-upcast to f32)
"""

import numpy as np

MODE = "bf16"

M, B, S, C, D = 4, 16, 512, 1024, 256
N_CORES = 8
B_LOC = B // N_CORES  # batches per core
KC = C // 128         # 8 c-chunks
KD = D // 128         # 2 d-chunks
KS = S // 128         # 4 s-chunks

_cache = {}
last_results = None  # BassKernelResults of the most recent run (for test.py)


def _build(mode):
    from contextlib import ExitStack

    import concourse.mybir as mybir
    import concourse.tile as tile
    from concourse import bacc

    f32 = mybir.dt.float32
    bf16 = mybir.dt.bfloat16
    in_dt = f32 if mode == "f32r" else bf16
    mm_dt = mybir.dt.float32r if mode == "f32r" else bf16
    out_dt = bf16 if mode == "bf16o" else f32

    nc = bacc.Bacc("TRN2", target_bir_lowering=False, debug=False,
                   num_devices=N_CORES)
    # xtp[b][p, k*512 + s] = x[b, s, 128k + p]   (x transposed, chunk-packed)
    xtp = nc.dram_tensor("xtp", [B_LOC, 128, KC * S], in_dt,
                         kind="ExternalInput").ap()
    # wpk[m, b][p, k*256 + d]        = down_w_gathered[m, b, 128k + p, d]
    # wpk[m, b][p, 2048 + j*1024+c]  = up_w_gathered[m, b, 128j + p, c]
    wpk = nc.dram_tensor("wpk", [M, B_LOC, 128, 4096], in_dt,
                         kind="ExternalInput").ap()
    # bdp[p, (m*B_LOC+b)*2 + j] = down_b_gathered[m, b, 128j + p]
    bdp = nc.dram_tensor("bdp", [128, M * B_LOC * KD], f32,
                         kind="ExternalInput").ap()
    out = nc.dram_tensor("out", [M, B_LOC, S, C], out_dt,
                         kind="ExternalOutput").ap()

    silu = mybir.ActivationFunctionType.Silu
    copy_fn = mybir.ActivationFunctionType.Copy

    def load(engine, dst, src):
        # casting DMA must go through SWDGE (gpsimd); plain DMA via HWDGE
        if mode == "f32r":
            nc.gpsimd.dma_start(dst, src)
        else:
            engine.dma_start(dst, src)

    with tile.TileContext(nc) as tc, ExitStack() as ctx:
        const = ctx.enter_context(tc.tile_pool(name="const", bufs=1))
        xpool = ctx.enter_context(tc.tile_pool(name="xpool", bufs=2))
        wpool = ctx.enter_context(tc.tile_pool(name="wpool", bufs=3))
        zpool = ctx.enter_context(tc.tile_pool(name="zpool", bufs=2))
        upool = ctx.enter_context(tc.tile_pool(name="upool", bufs=3))
        pszp = ctx.enter_context(tc.tile_pool(name="pszp", bufs=2, space="PSUM"))
        psup = ctx.enter_context(tc.tile_pool(name="psup", bufs=4, space="PSUM"))

        bd_sb = const.tile([128, M * B_LOC * KD], f32)
        nc.scalar.dma_start(bd_sb[:], bdp[:])

        mm_cast = (lambda ap: ap.bitcast(mm_dt)) if mode == "f32r" else (lambda ap: ap)

        for b in range(B_LOC):
            xt_sb = xpool.tile([128, KC * S], mm_dt)
            load(nc.sync, xt_sb[:], xtp[b])
            for m in range(M):
                w_sb = wpool.tile([128, 4096], mm_dt)
                load(nc.scalar, w_sb[:], wpk[m, b])

                z_sb = zpool.tile([128, KD, S], mm_dt)
                for j in range(KD):
                    psz = pszp.tile([128, S], f32)
                    for k in range(KC):
                        nc.tensor.matmul(
                            psz[:],
                            w_sb[:, k * 256 + j * 128: k * 256 + j * 128 + 128],
                            xt_sb[:, k * S: (k + 1) * S],
                            start=(k == 0), stop=(k == KC - 1),
                        )
                    col = (m * B_LOC + b) * KD + j
                    nc.scalar.activation(z_sb[:, j, :], psz[:], silu,
                                         bias=bd_sb[:, col: col + 1])

                u_sb = upool.tile([128, KS, C], out_dt)
                for a in range(KS):
                    for h in range(2):
                        psu = psup.tile([128, 512], f32)
                        for j in range(KD):
                            nc.tensor.matmul(
                                psu[:],
                                z_sb[:, j, a * 128: (a + 1) * 128],
                                w_sb[:, 2048 + j * 1024 + h * 512:
                                     2048 + j * 1024 + h * 512 + 512],
                                start=(j == 0), stop=(j == KD - 1),
                            )
                        dst = u_sb[:, a, h * 512: (h + 1) * 512]
                        if (a * 2 + h) % 2 == 0:
                            nc.vector.tensor_copy(dst, psu[:])
                        else:
                            nc.scalar.activation(dst, psu[:], copy_fn)
                nc.sync.dma_start(
                    out[m, b].rearrange("(a p) c -> p a c", p=128), u_sb[:])

    nc.compile()
    return nc


def _get_nc(mode):
    if mode not in _cache:
        _cache[mode] = _build(mode)
    return _cache[mode]


def kernel(x, expert_index, down_w, down_b, up_w):
    global last_results
    import ml_dtypes
    from concourse import bass_utils

    x = np.asarray(x, dtype=np.float32)
    idx = np.asarray(expert_index)
    r = np.arange(M)[:, None]
    wd = np.asarray(down_w, dtype=np.float32)[r, idx]   # [M, B, C, D]
    bd = np.asarray(down_b, dtype=np.float32)[r, idx]   # [M, B, D]
    wu = np.asarray(up_w, dtype=np.float32)[r, idx]     # [M, B, D, C]

    # Pack into SBUF partition-major layouts (see _build comments).
    xt = x.transpose(0, 2, 1).reshape(B, KC, 128, S)
    xt = xt.transpose(0, 2, 1, 3).reshape(B, 128, KC * S)
    wdp = wd.reshape(M, B, KC, 128, D).transpose(0, 1, 3, 2, 4)
    wdp = wdp.reshape(M, B, 128, KC * D)
    wup = wu.reshape(M, B, KD, 128, C).transpose(0, 1, 3, 2, 4)
    wup = wup.reshape(M, B, 128, KD * C)
    wpk = np.concatenate([wdp, wup], axis=-1)           # [M, B, 128, 4096]
    bdp = bd.reshape(M, B, KD, 128).transpose(3, 0, 1, 2)  # [128, M, B, KD]

    in_dt = np.float32 if MODE == "f32r" else ml_dtypes.bfloat16

    in_maps = []
    for i in range(N_CORES):
        bs = slice(i * B_LOC, (i + 1) * B_LOC)
        cols = bdp[:, :, bs, :].reshape(128, M * B_LOC * KD)
        in_maps.append({
            "xtp": np.ascontiguousarray(xt[bs].astype(in_dt)),
            "wpk": np.ascontiguousarray(wpk[:, bs].astype(in_dt)),
            "bdp": np.ascontiguousarray(cols.astype(np.float32)),
        })

    nc = _get_nc(MODE)
    res = bass_utils.run_bass_kernel_spmd(nc, in_maps,
                                          core_ids=list(range(N_CORES)))
    last_results = res

    full = np.empty((M, B, S, C), dtype=np.float32)
    for i in range(N_CORES):
        full[:, i * B_LOC:(i + 1) * B_LOC] = np.asarray(
            res.results[i]["out"]).astype(np.float32)
    return full


# revision 4
# speedup vs baseline: 1.3935x; 1.0066x over previous
"""Trainium2 Bass kernel for the AdapterController hard-routing MoE adapter.

Reference computation (per router m in [0,4), batch b in [0,16)):
    e = expert_index[m, b]
    z = x[b] @ down_w[m, e] + down_b[m, e]      # [512, 256]
    z = z * sigmoid(z)                          # swish
    u = z @ up_w[m, e]                          # [512, 1024]
    out[m, b] = u

Strategy: data-parallel over the batch axis (2 batches per core, 8 cores).
The expert gather is part of input sharding: each core receives exactly the
(m, b)-selected weight matrices, packed on the host into the SBUF partition
layout so every DMA is a single fully-contiguous >=1MB transfer.

On-chip per (m, b) pair:
    zT[d, s] = sum_c Wd[c, d] * xT[c, s]        (16 matmuls N=512, K=128)
    z = silu(zT + bd)                           (ACT engine, PSUM -> SBUF)
    u[s, c] = sum_d zT[d, s].T @ Wu[d, c]       (16 matmuls N=512)

MODE selects compute/transfer dtypes:
    "f32r":  f32 DMA, float32r matmuls (TF32-like, full PE rate)
    "bf16":  bf16 weights/x (host-cast), f32 output
    "bf16o": bf16 weights/x and bf16 output (host-upcast to f32)
"""

import numpy as np

MODE = "bf16o"

M, B, S, C, D = 4, 16, 512, 1024, 256
N_CORES = 8
B_LOC = B // N_CORES  # batches per core
KC = C // 128         # 8 c-chunks
KD = D // 128         # 2 d-chunks
KS = S // 128         # 4 s-chunks

_cache = {}
last_results = None  # BassKernelResults of the most recent run (for test.py)


def _build(mode):
    from contextlib import ExitStack

    import concourse.mybir as mybir
    import concourse.tile as tile
    from concourse import bacc

    f32 = mybir.dt.float32
    bf16 = mybir.dt.bfloat16
    in_dt = f32 if mode == "f32r" else bf16
    mm_dt = mybir.dt.float32r if mode == "f32r" else bf16
    out_dt = bf16 if mode == "bf16o" else f32

    nc = bacc.Bacc("TRN2", target_bir_lowering=False, debug=False,
                   num_devices=N_CORES)
    # xtp[b][p, k*512 + s] = x[b, s, 128k + p]   (x transposed, chunk-packed)
    xtp = nc.dram_tensor("xtp", [B_LOC, 128, KC * S], in_dt,
                         kind="ExternalInput").ap()
    # wpk[m, b][p, k*256 + d]        = down_w_gathered[m, b, 128k + p, d]
    # wpk[m, b][p, 2048 + j*1024+c]  = up_w_gathered[m, b, 128j + p, c]
    wpk = nc.dram_tensor("wpk", [M, B_LOC, 128, 4096], in_dt,
                         kind="ExternalInput").ap()
    # bdp[p, (m*B_LOC+b)*2 + j] = down_b_gathered[m, b, 128j + p]
    bdp = nc.dram_tensor("bdp", [128, M * B_LOC * KD], f32,
                         kind="ExternalInput").ap()
    out = nc.dram_tensor("out", [M, B_LOC, S, C], out_dt,
                         kind="ExternalOutput").ap()

    silu = mybir.ActivationFunctionType.Silu
    copy_fn = mybir.ActivationFunctionType.Copy

    def load(engine, dst, src):
        # casting DMA must go through SWDGE (gpsimd); plain DMA via HWDGE
        if mode == "f32r":
            nc.gpsimd.dma_start(dst, src)
        else:
            engine.dma_start(dst, src)

    with tile.TileContext(nc) as tc, ExitStack() as ctx:
        const = ctx.enter_context(tc.tile_pool(name="const", bufs=1))
        xpool = ctx.enter_context(tc.tile_pool(name="xpool", bufs=2))
        wpool = ctx.enter_context(tc.tile_pool(name="wpool", bufs=3))
        zpool = ctx.enter_context(tc.tile_pool(name="zpool", bufs=2))
        upool = ctx.enter_context(tc.tile_pool(name="upool", bufs=3))
        pszp = ctx.enter_context(tc.tile_pool(name="pszp", bufs=2, space="PSUM"))
        psup = ctx.enter_context(tc.tile_pool(name="psup", bufs=4, space="PSUM"))

        bd_sb = const.tile([128, M * B_LOC * KD], f32)
        nc.scalar.dma_start(bd_sb[:], bdp[:])

        mm_cast = (lambda ap: ap.bitcast(mm_dt)) if mode == "f32r" else (lambda ap: ap)

        for b in range(B_LOC):
            xt_sb = xpool.tile([128, KC * S], mm_dt)
            load(nc.sync, xt_sb[:], xtp[b])
            for m in range(M):
                w_sb = wpool.tile([128, 4096], mm_dt)
                load(nc.scalar, w_sb[:], wpk[m, b])

                z_sb = zpool.tile([128, KD, S], mm_dt)
                for j in range(KD):
                    psz = pszp.tile([128, S], f32)
                    for k in range(KC):
                        nc.tensor.matmul(
                            psz[:],
                            w_sb[:, k * 256 + j * 128: k * 256 + j * 128 + 128],
                            xt_sb[:, k * S: (k + 1) * S],
                            start=(k == 0), stop=(k == KC - 1),
                        )
                    col = (m * B_LOC + b) * KD + j
                    nc.scalar.activation(z_sb[:, j, :], psz[:], silu,
                                         bias=bd_sb[:, col: col + 1])

                u_sb = upool.tile([128, KS, C], out_dt)
                for a in range(KS):
                    for h in range(2):
                        psu = psup.tile([128, 512], f32)
                        for j in range(KD):
                            nc.tensor.matmul(
                                psu[:],
                                z_sb[:, j, a * 128: (a + 1) * 128],
                                w_sb[:, 2048 + j * 1024 + h * 512:
                                     2048 + j * 1024 + h * 512 + 512],
                                start=(j == 0), stop=(j == KD - 1),
                            )
                        dst = u_sb[:, a, h * 512: (h + 1) * 512]
                        if (a * 2 + h) % 2 == 0:
                            nc.vector.tensor_copy(dst, psu[:])
                        else:
                            nc.scalar.activation(dst, psu[:], copy_fn)
                nc.sync.dma_start(
                    out[m, b].rearrange("(a p) c -> p a c", p=128), u_sb[:])

    nc.compile()
    return nc


def _get_nc(mode):
    if mode not in _cache:
        _cache[mode] = _build(mode)
    return _cache[mode]


def kernel(x, expert_index, down_w, down_b, up_w):
    global last_results
    import ml_dtypes
    from concourse import bass_utils

    x = np.asarray(x, dtype=np.float32)
    idx = np.asarray(expert_index)
    r = np.arange(M)[:, None]
    wd = np.asarray(down_w, dtype=np.float32)[r, idx]   # [M, B, C, D]
    bd = np.asarray(down_b, dtype=np.float32)[r, idx]   # [M, B, D]
    wu = np.asarray(up_w, dtype=np.float32)[r, idx]     # [M, B, D, C]

    # Pack into SBUF partition-major layouts (see _build comments).
    xt = x.transpose(0, 2, 1).reshape(B, KC, 128, S)
    xt = xt.transpose(0, 2, 1, 3).reshape(B, 128, KC * S)
    wdp = wd.reshape(M, B, KC, 128, D).transpose(0, 1, 3, 2, 4)
    wdp = wdp.reshape(M, B, 128, KC * D)
    wup = wu.reshape(M, B, KD, 128, C).transpose(0, 1, 3, 2, 4)
    wup = wup.reshape(M, B, 128, KD * C)
    wpk = np.concatenate([wdp, wup], axis=-1)           # [M, B, 128, 4096]
    bdp = bd.reshape(M, B, KD, 128).transpose(3, 0, 1, 2)  # [128, M, B, KD]

    in_dt = np.float32 if MODE == "f32r" else ml_dtypes.bfloat16

    in_maps = []
    for i in range(N_CORES):
        bs = slice(i * B_LOC, (i + 1) * B_LOC)
        cols = bdp[:, :, bs, :].reshape(128, M * B_LOC * KD)
        in_maps.append({
            "xtp": np.ascontiguousarray(xt[bs].astype(in_dt)),
            "wpk": np.ascontiguousarray(wpk[:, bs].astype(in_dt)),
            "bdp": np.ascontiguousarray(cols.astype(np.float32)),
        })

    nc = _get_nc(MODE)
    res = bass_utils.run_bass_kernel_spmd(nc, in_maps,
                                          core_ids=list(range(N_CORES)))
    last_results = res

    full = np.empty((M, B, S, C), dtype=np.float32)
    for i in range(N_CORES):
        full[:, i * B_LOC:(i + 1) * B_LOC] = np.asarray(
            res.results[i]["out"]).astype(np.float32)
    return full
